# revision 1
# baseline (speedup 1.0000x reference)
"""2-layer GCN (GCNConv x2, relu between) on 8 Trainium2 NeuronCores.

v3 design (transform-first, bf16, interval-batched one-hot, split-pipelined
AllGathers):
  - Nodes partitioned into 8 contiguous shards by dst; each core owns the
    edges incident (by dst) to its shard. Self-loops are not tokens: they are
    added per window via an identity matmul.
  - Transform-first: stage0 computes h1 = dinv * (x @ W1) for the local shard
    from a host-pretransposed bf16 x^T; rows are stored 256B-padded so the
    AllGather output is directly dma_gather-able (elem_size must be a 256B
    multiple).
  - Each node's shard-local row r is assigned to part A (r < WA*128) or B;
    each layer runs phase A then phase B: gather tokens whose SOURCE is in
    that part (2.2 ns/desc, DMA-engine random-read bound), segment-sum via
    0/1 one-hot matmuls in bf16. Phase A accumulates into PSUM then parks in
    SBUF; phase B re-adds it in the tail. The part-k AllGather for phase k+1
    overlaps the previous phase's gathers:
        stage0(A) | AG1a ; stage0(B)
        L1-A gathers | AG1b
        L1-B gathers + tails | AG2a (after window 24's tail)
        L2-A gathers | AG2b
        L2-B gathers + final tails
  - One-hots are built batched (one DVE tensor_tensor per chunk, stride-0
    broadcast APs) against a [128, SPAN] iota; tokens are grouped into
    dst-intervals of width <= SPAN so each tile's one-hot spans SPAN columns.
  - Pads (dstv=-1) gather RANDOM rows: a fixed pad index would serialize all
    pad descriptors on one HBM channel.

All tile/interval structure is compile-time and shared across cores (from
max-over-cores token counts); per-core data differs only in gather indices
and dstv values.
"""

import sys

if "/opt/trn_rl_repo" not in sys.path:
    sys.path.insert(0, "/opt/trn_rl_repo")

import numpy as np
import ml_dtypes

P = 128
SPAN = 64
KMAX = 4
CHUNK_W = 6
GMAX = 8
NSWQ = 4

N, E, IN, HID, OUT = 50000, 800000, 128, 64, 32
N_CORES = 8
SH = N // N_CORES  # 6250
W = (SH + P - 1) // P  # 49
WA = 25  # windows in part A (8*RB must stay < 32768 for int16 idx)
RA = WA * P  # 3200 rows per shard in part A
RB = SH - RA  # 3050
NRA = N_CORES * RA  # 25600 (< 32768: int16 gather indices)
NRB = N_CORES * RB  # 24400
BF16 = ml_dtypes.bfloat16


def _cdiv(a, b):
    return (a + b - 1) // b


def preprocess(edge_index):
    """Host-side graph preprocessing -> (meta, per_core arrays)."""
    src = edge_index[0].astype(np.int64)
    dst = edge_index[1].astype(np.int64)

    deg = (np.bincount(dst, minlength=N) + 1.0).astype(np.float32)
    dinv = (1.0 / np.sqrt(deg)).astype(np.float32)

    core = dst // SH
    dloc = dst % SH
    wv = dloc >> 7
    dl = dloc & 127
    # source part + part-local gather row
    s_core = src // SH
    s_r = src % SH
    g = (s_r >= RA).astype(np.int64)  # 0 = part A, 1 = part B
    s_row = np.where(g == 0, s_core * RA + s_r, s_core * RB + (s_r - RA))

    counts = np.zeros((N_CORES, W, 2, P), np.int64)
    np.add.at(counts, (core, wv, g, dl), 1)

    # shared interval structure per (w, g)
    intervals = {}
    for w in range(W):
        for gg in (0, 1):
            cnt = counts[:, w, gg, :]  # [8, 128]
            ivs = []
            d = 0
            while d < P:
                while d < P and cnt[:, d].sum() == 0:
                    d += 1
                if d >= P:
                    break
                d0e = min(d & ~1, P - SPAN)
                acc = np.zeros(N_CORES, np.int64)
                end = d
                while end < P and end < d0e + SPAN:
                    na = acc + cnt[:, end]
                    if na.max() > KMAX * P:
                        break
                    acc = na
                    end += 1
                ivs.append((d, end, d0e, int(_cdiv(int(acc.max()), P))))
                d = end
            intervals[(w, gg)] = ivs

    # tile table: PHASE-major (all part-A chunks, then all part-B chunks)
    tiles = []
    chunks = []  # per chunk: t0, t1, ws, part
    win_tiles = {(w, gg): [] for w in range(W) for gg in (0, 1)}
    for gg in (0, 1):
        for c0 in range(0, W, CHUNK_W):
            ws = list(range(c0, min(c0 + CHUNK_W, W)))
            ch = {"t0": len(tiles), "ws": ws, "part": gg}
            for w in ws:
                for d0s, end, d0e, nt in intervals[(w, gg)]:
                    for k in range(nt):
                        tiles.append({"w": w, "g": gg, "d0": d0e,
                                      "lo": d0s, "hi": end, "slot": k})
                        win_tiles[(w, gg)].append(len(tiles) - 1)
            ch["t1"] = len(tiles)
            if ch["t1"] > ch["t0"]:
                chunks.append(ch)
    NT = len(tiles)

    # per-core token arrays
    key_all = ((wv * 2 + g) * P + dl)
    per_core = []
    for c in range(N_CORES):
        sel = core == c
        key_c = key_all[sel]
        row_c = s_row[sel]
        dl_c = dl[sel]
        order = np.argsort(key_c, kind="stable")
        key_s = key_c[order]
        idx_s = row_c[order]
        dl_s = dl_c[order]

        # pads (dstv=-1) contribute zero but still issue one descriptor each;
        # spread their reads over the table so they don't hammer one HBM
        # channel (a fixed pad index serializes on a single 256B row).
        rng = np.random.default_rng(1234 + c)
        tok_idx = rng.integers(0, min(NRA, NRB), size=NT * P).astype(np.int32)
        tok_dstv = np.full(NT * P, -1.0, np.float32)
        for w in range(W):
            for gg in (0, 1):
                base_key = (w * 2 + gg) * P
                tlist = win_tiles[(w, gg)]
                i = 0
                while i < len(tlist):
                    t0 = tlist[i]
                    lo_dl, hi_dl, d0e = (tiles[t0]["lo"], tiles[t0]["hi"],
                                         tiles[t0]["d0"])
                    ntk = 1
                    while (i + ntk < len(tlist)
                           and tiles[tlist[i + ntk]]["lo"] == lo_dl
                           and tiles[tlist[i + ntk]]["slot"] == ntk):
                        ntk += 1
                    a = np.searchsorted(key_s, base_key + lo_dl)
                    b = np.searchsorted(key_s, base_key + hi_dl)
                    n_c = b - a
                    for k in range(ntk):
                        tid = tlist[i + k]
                        s0 = a + k * P
                        m = max(0, min(P, n_c - k * P))
                        if m > 0:
                            o = tid * P
                            tok_idx[o : o + m] = idx_s[s0 : s0 + m]
                            tok_dstv[o : o + m] = (dl_s[s0 : s0 + m] - d0e)
                    i += ntk

        assert tok_idx.max() < 32768 and tok_idx.min() >= 0
        i16 = tok_idx.astype(np.int16).reshape(NT * P // 16, 16).T
        i16 = np.tile(i16, (8, 1)).copy()  # [128, NT*8]
        dstv_sb = tok_dstv.reshape(NT, P).T.astype(BF16).copy()  # [128, NT]

        dpad = np.ones(W * P, np.float32)
        dpad[:SH] = dinv[c * SH : (c + 1) * SH]
        dinvbc = np.tile(dpad[None, :], (P, 1)).copy()  # [128, W*128] f32
        dinvw = dpad.reshape(W, P).T.copy()  # [128, W] f32

        per_core.append({"src16": i16, "dstv": dstv_sb,
                         "dinvbc": dinvbc, "dinvw": dinvw})

    meta = {"NT": NT, "tiles": tiles, "chunks": chunks,
            "win_tiles": win_tiles}
    return meta, per_core


IN_NAMES = ["xT", "w1b", "w2b", "b1col", "b2bc", "iota", "i128b",
            "dinvw", "dinvbc", "src16", "dstv"]


def make_inputs(x, W1, b1, W2, b2, meta, per_core):
    iota = np.tile(np.arange(SPAN, dtype=np.float32)[None, :], (P, 1)).astype(BF16)
    i128b = np.eye(P, dtype=np.float32).astype(BF16)
    b1col = np.asarray(b1, np.float32).reshape(HID, 1)
    b2bc = np.tile(np.asarray(b2, np.float32)[None, :], (P, 1)).astype(BF16)
    w1b = np.asarray(W1, np.float32).astype(BF16)
    w2b = np.asarray(W2, np.float32).astype(BF16)
    xf = np.asarray(x, np.float32)
    ins_list = []
    for c, pc in enumerate(per_core):
        xT = np.zeros((P, W * P), np.float32)
        xT[:, :SH] = xf[c * SH : (c + 1) * SH, :].T
        ins_list.append([
            xT.astype(BF16),
            w1b,
            w2b,
            b1col,
            b2bc,
            iota,
            i128b,
            pc["dinvw"],
            pc["dinvbc"],
            pc["src16"],
            pc["dstv"],
        ])
    return ins_list


def build_kernel(tc, outs, ins, meta):
    from concourse import mybir

    nc = tc.nc
    (xT_ap, w1_ap, w2_ap, b1_ap, b2_ap, iota_ap, i128b_ap,
     dinvw_ap, dinvbc_ap, src16_ap, dstv_ap) = ins
    out_ap = outs[0]

    NT = meta["NT"]
    tiles = meta["tiles"]
    chunks = meta["chunks"]
    win_tiles = meta["win_tiles"]
    groups = [list(range(N_CORES))]

    f32 = mybir.dt.float32
    bf16 = mybir.dt.bfloat16
    i16 = mybir.dt.int16
    AT = mybir.ActivationFunctionType
    OP = mybir.AluOpType

    MAX_CT = max(ch["t1"] - ch["t0"] for ch in chunks)

    import contextlib

    with contextlib.ExitStack() as ctx:
        const = ctx.enter_context(tc.tile_pool(name="const", bufs=1))
        dram = ctx.enter_context(tc.tile_pool(name="dram", bufs=1, space="DRAM"))
        ohp = ctx.enter_context(tc.tile_pool(name="oh", bufs=4))
        tokp = ctx.enter_context(tc.tile_pool(name="tokp", bufs=3))
        tailp = ctx.enter_context(tc.tile_pool(name="tail", bufs=4))
        persist = ctx.enter_context(tc.tile_pool(name="persist", bufs=1))

        # constants
        w1_sb = const.tile([IN, HID], bf16)
        nc.sync.dma_start(w1_sb[:], w1_ap[:])
        w2_sb = const.tile([HID, OUT], bf16)
        nc.sync.dma_start(w2_sb[:], w2_ap[:])
        b1_sb = const.tile([HID, 1], f32)
        nc.sync.dma_start(b1_sb[:], b1_ap[:])
        b2_sb = const.tile([P, OUT], bf16)
        nc.sync.dma_start(b2_sb[:], b2_ap[:])
        iota_sb = const.tile([P, SPAN], bf16)
        nc.sync.dma_start(iota_sb[:], iota_ap[:])
        i128b_sb = const.tile([P, P], bf16)
        nc.sync.dma_start(i128b_sb[:], i128b_ap[:])
        dinvw_sb = const.tile([P, W], f32)
        nc.sync.dma_start(dinvw_sb[:], dinvw_ap[:])
        dinvbc_sb = const.tile([P, W * P], f32)
        nc.sync.dma_start(dinvbc_sb[:], dinvbc_ap[:])
        src16_sb = const.tile([P, NT * 8], i16)
        nc.sync.dma_start(src16_sb[:], src16_ap[:])
        dstv_sb = const.tile([P, NT], bf16)
        nc.sync.dma_start(dstv_sb[:], dstv_ap[:])
        zrow_sb = const.tile([1, P], bf16)
        nc.vector.memset(zrow_sb[:], 0.0)

        # persistent per-window row tiles (pre-scaled bf16)
        h1p_sb = persist.tile([P, W, HID], bf16)
        h2p_sb = persist.tile([P, W, OUT], bf16)

        # DRAM scratch (rows padded to 256B for dma_gather)
        h1_shardA = dram.tile([RA, P], bf16)
        h1_shardB = dram.tile([RB, P], bf16)
        h1xA = dram.tile([NRA, P], bf16)
        h1xB = dram.tile([NRB, P], bf16)
        h2_shardA = dram.tile([RA, P], bf16)
        h2_shardB = dram.tile([RB, P], bf16)
        h2xA = dram.tile([NRA, P], bf16)
        h2xB = dram.tile([NRB, P], bf16)
        warm_in = dram.tile([1, P], bf16)
        warm_out = dram.tile([N_CORES, P], bf16)

        qctr = [0]

        # warm up the collective stream (absorbs the one-time ~30us barrier
        # while stage0 runs)
        nc.gpsimd.collective_compute(
            "AllGather", mybir.AluOpType.bypass, replica_groups=groups,
            ins=[warm_in[:]], outs=[warm_out[:]],
        )

        # ---------------- stage 0: h1 = dinv * (x @ W1), bf16 ----------------
        with tc.tile_pool(name="stage0ps", bufs=3, space="PSUM") as ps0, \
             tc.tile_pool(name="xload", bufs=1) as xp:
            xT_sb = xp.tile([P, W * P], bf16)
            nc.sync.dma_start(xT_sb[:], xT_ap[:])

            def stage0_win(w):
                dw = min(P, SH - w * P)
                h1_ps = ps0.tile([P, HID], f32, tag="h1")
                nc.tensor.matmul(
                    out=h1_ps[:dw, :],
                    lhsT=xT_sb[:, w * P : w * P + dw],
                    rhs=w1_sb[:],
                    start=True, stop=True,
                )
                nc.scalar.activation(
                    out=h1p_sb[:dw, w, :], in_=h1_ps[:dw, :], func=AT.Copy,
                    scale=dinvw_sb[:dw, w : w + 1],
                )
                shard = h1_shardA if w < WA else h1_shardB
                r0 = w * P if w < WA else w * P - RA
                nc.sync.dma_start(shard[r0 : r0 + dw, 0:HID],
                                  h1p_sb[:dw, w, :])

            for w in range(WA):
                stage0_win(w)
            nc.gpsimd.collective_compute(
                "AllGather", mybir.AluOpType.bypass, replica_groups=groups,
                ins=[h1_shardA[:]], outs=[h1xA[:]],
            )
            for w in range(WA, W):
                stage0_win(w)
            nc.gpsimd.collective_compute(
                "AllGather", mybir.AluOpType.bypass, replica_groups=groups,
                ins=[h1_shardB[:]], outs=[h1xB[:]],
            )

        psum_s = ctx.enter_context(tc.tile_pool(name="psum_s", bufs=4, space="PSUM"))
        psum_g = ctx.enter_context(tc.tile_pool(name="psum_g", bufs=2, space="PSUM"))

        def tail(w, dw, SB, SA_sb, is_l1):
            F = HID if is_l1 else OUT
            t0v = tailp.tile([HID, P], f32, tag="t0", name=f"t0_{w}")
            nc.vector.tensor_tensor(
                out=t0v[:F, :], in0=SB[:, :], in1=SA_sb[:F, w * P : (w + 1) * P],
                op=OP.add,
            )
            if is_l1:
                t1 = tailp.tile([HID, P], f32, tag="t1")
                nc.vector.tensor_tensor(
                    out=t1[:], in0=t0v[:HID, :],
                    in1=dinvbc_sb[:HID, w * P : (w + 1) * P], op=OP.mult,
                )
                ut = tailp.tile([HID, P], bf16, tag="ut")
                nc.scalar.activation(
                    out=ut[:], in_=t1[:], func=AT.Relu, bias=b1_sb[:, 0:1],
                )
                h2T_ps = psum_g.tile([OUT, P], f32, tag="h2T")
                nc.tensor.matmul(
                    out=h2T_ps[:], lhsT=w2_sb[:], rhs=ut[:],
                    start=True, stop=True,
                )
                h2T_sb = tailp.tile([OUT, P], bf16, tag="h2Ts")
                nc.vector.tensor_tensor(
                    out=h2T_sb[:], in0=h2T_ps[:],
                    in1=dinvbc_sb[:OUT, w * P : (w + 1) * P], op=OP.mult,
                )
                h2p_ps = psum_g.tile([P, OUT], bf16, tag="h2p")
                nc.tensor.transpose(
                    h2p_ps[:], h2T_sb[:], i128b_sb[:OUT, :OUT]
                )
                nc.scalar.activation(
                    out=h2p_sb[:dw, w, :], in_=h2p_ps[:dw, :], func=AT.Copy,
                )
                shard = h2_shardA if w < WA else h2_shardB
                r0 = w * P if w < WA else w * P - RA
                nc.sync.dma_start(shard[r0 : r0 + dw, 0:OUT],
                                  h2p_sb[:dw, w, :])
            else:
                o1 = tailp.tile([OUT, P], bf16, tag="o1")
                nc.vector.tensor_tensor(
                    out=o1[:], in0=t0v[:OUT, :],
                    in1=dinvbc_sb[:OUT, w * P : (w + 1) * P], op=OP.mult,
                )
                o2_ps = psum_g.tile([P, OUT], bf16, tag="h2p")
                nc.tensor.transpose(o2_ps[:], o1[:], i128b_sb[:OUT, :OUT])
                o3 = tailp.tile([P, OUT], f32, tag="o3")
                nc.vector.tensor_tensor(
                    out=o3[:dw, :], in0=o2_ps[:dw, :], in1=b2_sb[:dw, :],
                    op=OP.add,
                )
                nc.sync.dma_start(out_ap[w * P : w * P + dw, :], o3[:dw, :])

        def phase(part, src_hx, F, row_sb, SA_sb, is_l1, post_win=None):
            """One gather/segment-sum phase over one source part."""
            for ch in chunks:
                if ch["part"] != part:
                    continue
                t0 = ch["t0"]
                ct = ch["t1"] - t0
                tokt = tokp.tile([P, MAX_CT, P], bf16, tag="tok")
                for ca in range(t0, ch["t1"], GMAX):
                    cb = min(ca + GMAX, ch["t1"])
                    nc.gpsimd.dma_gather(
                        out_ap=tokt[:, ca - t0 : cb - t0, :],
                        in_ap=src_hx[:, :],
                        idxs_ap=src16_sb[:, ca * 8 : cb * 8],
                        num_idxs=(cb - ca) * P,
                        num_idxs_reg=(cb - ca) * P,
                        elem_size=P,
                        single_packet=True,
                        queue_num=qctr[0] % NSWQ,
                    )
                    qctr[0] += 1
                oh = ohp.tile([P, MAX_CT, SPAN], bf16, tag="oh")
                nc.vector.tensor_tensor(
                    out=oh[:, :ct, :],
                    in0=dstv_sb[:, t0 : t0 + ct].unsqueeze(2)
                        .broadcast_to([P, ct, SPAN]),
                    in1=iota_sb[:].unsqueeze(1).broadcast_to([P, ct, SPAN]),
                    op=OP.is_equal,
                )
                for w in ch["ws"]:
                    dw = min(P, SH - w * P)
                    wt = win_tiles[(w, part)]
                    S_t = psum_s.tile([HID, P], f32, tag="S",
                                      name=f"S_{part}_{w}")
                    S = S_t[:F, :]
                    if part == 0:
                        # zero-init via 1-row zero matmul
                        nc.tensor.matmul(
                            out=S[:, :], lhsT=zrow_sb[:, 0:F],
                            rhs=zrow_sb[:, 0:P],
                            start=True, stop=(len(wt) == 0),
                            skip_group_check=True,
                        )
                    else:
                        # self-loop rows (pre-scaled) double as zero-init
                        nc.tensor.matmul(
                            out=S[:, :],
                            lhsT=row_sb[:dw, w, :],
                            rhs=i128b_sb[:dw, :],
                            start=True, stop=(len(wt) == 0),
                            skip_group_check=True,
                        )
                    for j, tid in enumerate(wt):
                        tm = tiles[tid]
                        d0 = tm["d0"]
                        nc.tensor.matmul(
                            out=S[:, d0 : d0 + SPAN],
                            lhsT=tokt[:, tid - t0, 0:F],
                            rhs=oh[:, tid - t0, :],
                            start=False, stop=(j == len(wt) - 1),
                            skip_group_check=True,
                        )
                    if part == 0:
                        # park phase-A partials in SBUF
                        nc.scalar.activation(
                            out=SA_sb[:F, w * P : (w + 1) * P], in_=S[:, :],
                            func=AT.Copy,
                        )
                    else:
                        tail(w, dw, S, SA_sb, is_l1)
                if post_win is not None:
                    post_win(ch["ws"][-1])

        def layer(hxA, hxB, F, row_sb, is_l1, post_win=None):
            with tc.tile_pool(name=f"sa{1 if is_l1 else 2}", bufs=1) as sap:
                SA_sb = sap.tile([F, W * P], f32)
                phase(0, hxA, F, row_sb, SA_sb, is_l1)
                phase(1, hxB, F, row_sb, SA_sb, is_l1, post_win=post_win)

        # ---------------- L1 ----------------
        ag2a_done = [False]

        def l1_post(last_w):
            if last_w >= WA - 1 and not ag2a_done[0]:
                ag2a_done[0] = True
                nc.gpsimd.collective_compute(
                    "AllGather", mybir.AluOpType.bypass, replica_groups=groups,
                    ins=[h2_shardA[:]], outs=[h2xA[:]],
                )

        layer(h1xA, h1xB, HID, h1p_sb, True, post_win=l1_post)
        nc.gpsimd.collective_compute(
            "AllGather", mybir.AluOpType.bypass, replica_groups=groups,
            ins=[h2_shardB[:]], outs=[h2xB[:]],
        )

        # ---------------- L2 ----------------
        layer(h2xA, h2xB, OUT, h2p_sb, False)


def compile_kernel(x, W1, b1, W2, b2, edge_index):
    """Build + compile. Returns (nc, in_maps, meta)."""
    import concourse.tile as tile
    from concourse import bacc, mybir

    meta, per_core = preprocess(np.asarray(edge_index))
    ins_list = make_inputs(x, W1, b1, W2, b2, meta, per_core)

    nc = bacc.Bacc(
        "TRN2", target_bir_lowering=False, debug=False, num_devices=N_CORES,
        num_swdge_queues=NSWQ,
    )
    in_aps = []
    for nm, a in zip(IN_NAMES, ins_list[0]):
        in_aps.append(
            nc.dram_tensor(nm, list(a.shape), mybir.dt.from_np(a.dtype),
                           kind="ExternalInput").ap()
        )
    out_t = nc.dram_tensor("out", [SH, OUT], mybir.dt.float32,
                           kind="ExternalOutput")
    with tile.TileContext(nc) as tc:
        build_kernel(tc, [out_t.ap()], in_aps, meta)
    nc.compile()

    in_maps = [
        {nm: np.ascontiguousarray(a) for nm, a in zip(IN_NAMES, arrs)}
        for arrs in ins_list
    ]
    return nc, in_maps, meta


def run(x, W1, b1, W2, b2, edge_index, trace=False, ntff=False, tmpdir=None):
    from concourse import bass_utils
    from concourse.bass_interp import get_hw_module

    nc, in_maps, meta = compile_kernel(x, W1, b1, W2, b2, edge_index)
    old_m = nc.m
    nc.m = get_hw_module(nc.m)
    try:
        res = bass_utils.run_bass_kernel_spmd(
            nc, in_maps, core_ids=list(range(N_CORES)), trace=ntff,
            tmpdir=tmpdir,
        )
        bench_ns = _bench(nc, in_maps, N_CORES) if trace else None
    finally:
        nc.m = old_m
    out = np.concatenate([res.results[c]["out"] for c in range(N_CORES)], axis=0)
    return out, res, bench_ns


def _bench(nc, in_maps, n_cores, iters=30):
    """Interleaved wall-clock benchmark (upper bound on HW time)."""
    import time

    import jax
    from concourse import bass2jax
    from jax.sharding import Mesh, PartitionSpec
    from jax.experimental.shard_map import shard_map

    part_name = nc.partition_id_tensor.name if nc.partition_id_tensor else None
    in_names, out_names, out_avals, zero_outs = [], [], [], []
    for alloc in nc.m.functions[0].allocations:
        if not isinstance(alloc, bass2jax.mybir.MemoryLocationSet):
            continue
        name = alloc.memorylocations[0].name
        if alloc.kind == "ExternalInput":
            if name != part_name:
                in_names.append(name)
        elif alloc.kind == "ExternalOutput":
            out_names.append(name)
            shape = tuple(alloc.tensor_shape)
            dtype = bass2jax.mybir.dt.np(alloc.dtype)
            out_avals.append(jax.core.ShapedArray(shape, dtype))
            zero_outs.append(np.zeros(shape, dtype))
    n_params = len(in_names)
    all_names = in_names + out_names
    if part_name is not None:
        all_names = all_names + [part_name]

    def _body(*args):
        ins = list(args[:n_params])
        outs = list(args[n_params:])
        operands = ins + outs
        if part_name is not None:
            operands.append(bass2jax.partition_id_tensor())
        outs = list(
            bass2jax._bass_exec_p.bind(
                *operands,
                out_avals=tuple(out_avals),
                in_names=tuple(all_names),
                out_names=tuple(out_names),
                lowering_input_output_aliases=(),
                sim_require_finite=True,
                sim_require_nnan=True,
                nc=nc,
            )
        )
        return tuple(outs)

    devices = jax.devices()[:n_cores]
    mesh = Mesh(np.asarray(devices), ("core",))
    nio = n_params + len(out_names)
    sh = jax.sharding.NamedSharding(mesh, PartitionSpec("core"))
    concat_in = [
        jax.device_put(
            np.concatenate([in_maps[c][nm] for c in range(n_cores)], axis=0), sh
        )
        for nm in in_names
    ]
    concat_zero = [
        jax.device_put(np.zeros((n_cores * z.shape[0], *z.shape[1:]), z.dtype), sh)
        for z in zero_outs
    ]

    fn = jax.jit(
        shard_map(
            _body,
            mesh=mesh,
            in_specs=(PartitionSpec("core"),) * nio,
            out_specs=(PartitionSpec("core"),) * len(out_names),
            check_rep=False,
        ),
        keep_unused=True,
    )
    base_fn = jax.jit(lambda a: a[0:1, 0:1] * 2.0)
    jax.block_until_ready(fn(*concat_in, *concat_zero))
    jax.block_until_ready(base_fn(concat_in[0]))
    deltas = []
    for _ in range(iters):
        t0 = time.perf_counter()
        jax.block_until_ready(base_fn(concat_in[0]))
        t1 = time.perf_counter()
        jax.block_until_ready(fn(*concat_in, *concat_zero))
        t2 = time.perf_counter()
        jax.block_until_ready(base_fn(concat_in[0]))
        t3 = time.perf_counter()
        deltas.append((t2 - t1) - ((t1 - t0) + (t3 - t2)) / 2.0)
    deltas.sort()
    med = deltas[len(deltas) // 2]
    print(f"[bench] interleaved delta min={deltas[0]*1e6:.1f}us "
          f"median={med*1e6:.1f}us max={deltas[-1]*1e6:.1f}us")
    return int(max(0.0, med) * 1e9)


def kernel(x, W1, b1, W2, b2, edge_index):
    out, _, _ = run(
        np.asarray(x, np.float32),
        np.asarray(W1, np.float32),
        np.asarray(b1, np.float32),
        np.asarray(W2, np.float32),
        np.asarray(b2, np.float32),
        np.asarray(edge_index, np.int32),
    )
    return out

